# revision 7
# baseline (speedup 1.0000x reference)
"""Trainium2 Bass kernel for CustomImageCaptionModel.

CNN encoder (4x conv3x3+BN+ReLU+pool) -> FC -> [ctx|emb] -> LSTM(T=24) ->
vocab projection.  8-core SPMD:
  - convs data-parallel over batch (2 images/core)
  - FC K-sharded (AllToAll features, AllReduce partials)
  - LSTM gate-sharded (512 gate-dims/core, per-step AllGather of h)
  - vocab projection sharded over vocab (1250 rows/core, batched over tokens)
"""
import sys
sys.path.insert(0, '/opt/trn_rl_repo')

import numpy as np
import ml_dtypes

import concourse.bass as bass
import concourse.bacc as bacc
import concourse.tile as tile
import concourse.mybir as mybir
from concourse import bass_utils

BF = ml_dtypes.bfloat16
F32 = mybir.dt.float32
BF16 = mybir.dt.bfloat16
AF = mybir.ActivationFunctionType
ALU = mybir.AluOpType

R = 8            # cores
BL = 2           # images per core
T = 24
B = 16
GPERM = [0, 1, 3, 2]   # gate order [i, f, o, g] (reference order i,f,g,o)

DEBUG_OUTS = False

_cache = {}


# --------------------------------------------------------------------------
# device program
# --------------------------------------------------------------------------
def build_program():
    nc = bacc.Bacc("TRN2", target_bir_lowering=False, debug=False,
                   enable_asserts=True, num_devices=R)

    d = {}
    def din(name, shape, dt):
        d[name] = nc.dram_tensor(name, list(shape), dt, kind="ExternalInput")
        return d[name]

    imgs = din("imgs", [BL, 3, 226, 226], BF16)
    w1bd = din("w1bd", [108, 128], BF16)
    bc1 = din("bc1", [128, 1], F32)
    w2k = din("w2k", [3, 96, 64], BF16)
    bc2 = din("bc2", [64, 1], F32)
    w3k = din("w3k", [576, 128], BF16)
    bc3 = din("bc3", [128, 1], F32)
    w4k = din("w4k", [1152, 256], BF16)
    bc4 = din("bc4", [128, 2], F32)
    fcw = din("fcw", [49, 128, 512], BF16)
    fcb = din("fcb", [128, 4], F32)
    xh = din("xh", [1024, 384], BF16)
    wih = din("wih", [48, 128, 128], BF16)
    bsum = din("bsum", [128, 4], F32)
    whh = din("whh", [32, 128, 128], BF16)
    outw = din("outw", [80, 128, 128], BF16)
    outb = din("outb", [128, 10], F32)

    logits = nc.dram_tensor("logits", [10, 128, 384], F32, kind="ExternalOutput")
    if DEBUG_OUTS:
        dbg_if = nc.dram_tensor("dbg_if", [128, 4, 16], F32, kind="ExternalOutput")
        dbg_h = nc.dram_tensor("dbg_h", [128, 8, 16], F32, kind="ExternalOutput")
        dbg_feat = nc.dram_tensor("dbg_feat", [2, 128, 196], F32,
                                  kind="ExternalOutput")

    with tile.TileContext(nc) as tc:
        with (
            tc.tile_pool(name="res", bufs=1) as res,
            tc.tile_pool(name="dram", bufs=1, space="DRAM") as dpool,
            tc.tile_pool(name="agd", bufs=2, space="DRAM") as agd,
        ):
            # ---- resident weights ----
            w1sb = res.tile([108, 128], BF16)
            nc.sync.dma_start(w1sb[:], w1bd[:])
            w2sb = res.tile([96, 3, 64], BF16)
            nc.sync.dma_start(w2sb[:], w2k[:].transpose([1, 0, 2]))
            w3sb = res.tile([128, 5, 128], BF16)
            for q in range(5):
                kq = 128 if q < 4 else 64
                nc.sync.dma_start(w3sb[:kq, q, :], w3k[128 * q:128 * q + kq, :])
            w4sb = res.tile([128, 9, 256], BF16)
            nc.sync.dma_start(w4sb[:], w4k[:].rearrange("(t p) m -> p t m", p=128))
            wihsb = res.tile([128, 48, 128], BF16)
            nc.sync.dma_start(wihsb[:], wih[:].transpose([1, 0, 2]))
            whhsb = res.tile([128, 32, 128], BF16)
            nc.sync.dma_start(whhsb[:], whh[:].transpose([1, 0, 2]))
            outwsb = res.tile([128, 80, 128], BF16)
            nc.sync.dma_start(outwsb[:], outw[:].transpose([1, 0, 2]))
            bc1s = res.tile([128, 1], F32); nc.sync.dma_start(bc1s[:], bc1[:])
            bc2s = res.tile([64, 1], F32);  nc.sync.dma_start(bc2s[:], bc2[:])
            bc3s = res.tile([128, 1], F32); nc.sync.dma_start(bc3s[:], bc3[:])
            bc4s = res.tile([128, 2], F32); nc.sync.dma_start(bc4s[:], bc4[:])
            fcbs = res.tile([128, 4], F32); nc.sync.dma_start(fcbs[:], fcb[:])
            bsums = res.tile([128, 4], F32); nc.sync.dma_start(bsums[:], bsum[:])
            outbs = res.tile([128, 10], F32); nc.sync.dma_start(outbs[:], outb[:])

            # ---- persistent activations ----
            xT = res.tile([128, 12, 384], BF16)       # X^T k-tiles
            nc.sync.dma_start(xT[:, 4:12, :], xh[:].rearrange("(k p) n -> p k n", p=128))
            gx = res.tile([128, 24, 64], F32)         # precomputed x-gates (+bias)
            lstm_buf = res.tile([128, 8, 384], BF16)  # h^T for vocab projection
            feat = [[None] * BL for _ in range(2)]    # conv4 pooled output tiles

            # ============ conv encoder (per local image) ============
            with (
                tc.tile_pool(name="cwork", bufs=2) as cw,
                tc.tile_pool(name="cbig", bufs=1) as cb,
                tc.tile_pool(name="cpsum", bufs=8, space="PSUM") as cps,
            ):
                for i in range(BL):
                    # ---- conv1: [3,224,224] -> [32,112,112] ----
                    ic1 = cb.tile([108, 56, 224], BF16, tag="ic1")
                    icv = ic1[:].rearrange("(s j) h w -> s j h w", s=4)
                    img = imgs[i]
                    for ky in range(3):
                        for kx in range(3):
                            j0 = ky * 9 + kx * 3
                            for s in range(4):
                                src = img[:, 56 * s + ky:56 * s + ky + 56,
                                          kx:kx + 224]
                                nc.sync.dma_start(icv[s, j0:j0 + 3], src)
                    pr1 = cb.tile([128, 28, 112], BF16, tag="pr1")
                    for k in range(28):
                        ps = cps.tile([128, 448], F32, tag="cps")
                        nc.tensor.matmul(ps[:], w1sb[:], ic1[:, 2 * k:2 * k + 2, :],
                                         start=True, stop=True)
                        ev = cw.tile([128, 224], F32, tag="ev1")
                        nc.scalar.activation(ev[:], ps[:, 0::2], AF.Copy)
                        wm = cw.tile([128, 224], F32, tag="wm1")
                        nc.vector.tensor_tensor(wm[:], ev[:], ps[:, 1::2],
                                                op=ALU.max)
                        hm = cw.tile([128, 112], F32, tag="hm1")
                        nc.vector.tensor_tensor(hm[:], wm[:, 0:112], wm[:, 112:224],
                                                op=ALU.max)
                        nc.scalar.activation(pr1[:, k, :], hm[:], AF.Relu,
                                             bias=bc1s[:])
                    # shuffle strips -> p1 [32 ch, 114, 114] padded
                    p1 = cb.tile([32, 114, 114], BF16, tag="p1")
                    nc.vector.memset(p1[:, 0, :], 0.0)
                    nc.vector.memset(p1[:, 113, :], 0.0)
                    nc.vector.memset(p1[:, :, 0], 0.0)
                    nc.vector.memset(p1[:, :, 113], 0.0)
                    for s in range(4):
                        nc.sync.dma_start(p1[:, 28 * s + 1:28 * s + 29, 1:113],
                                          pr1[32 * s:32 * s + 32, :, :])

                    # ---- conv2: [32,112,112] -> [64,56,56] ----
                    p2 = cb.tile([64, 58, 58], BF16, tag="p2")
                    nc.vector.memset(p2[:, 0, :], 0.0)
                    nc.vector.memset(p2[:, 57, :], 0.0)
                    nc.vector.memset(p2[:, :, 0], 0.0)
                    nc.vector.memset(p2[:, :, 57], 0.0)
                    for c in range(7):
                        ics = [cw.tile([96, 16, 112], BF16, tag=f"ic2_{q}", name=f"ic2_{q}")
                               for q in range(3)]
                        for kk in range(9):
                            ky, kx = kk // 3, kk % 3
                            q, g = kk // 3, kk % 3
                            # note: q = ky, g = kx with this grouping
                            nc.sync.dma_start(
                                ics[q][32 * g:32 * g + 32, :, :],
                                p1[:, ky + 16 * c:ky + 16 * c + 16, kx:kx + 112])
                        for n in range(8):
                            ps = cps.tile([64, 224], F32, tag="cps")
                            for q in range(3):
                                nc.tensor.matmul(ps[:], w2sb[:, q, :],
                                                 ics[q][:, 2 * n:2 * n + 2, :],
                                                 start=(q == 0), stop=(q == 2))
                            ev = cw.tile([64, 112], F32, tag="ev2")
                            nc.scalar.activation(ev[:], ps[:, 0::2], AF.Copy)
                            wm = cw.tile([64, 112], F32, tag="wm2")
                            nc.vector.tensor_tensor(wm[:], ev[:], ps[:, 1::2],
                                                    op=ALU.max)
                            hm = cw.tile([64, 56], F32, tag="hm2")
                            nc.vector.tensor_tensor(hm[:], wm[:, 0:56], wm[:, 56:112],
                                                    op=ALU.max)
                            nc.scalar.activation(p2[:, 1 + 8 * c + n, 1:57], hm[:],
                                                 AF.Relu, bias=bc2s[:])

                    # ---- conv3: [64,56,56] -> [128,28,28] ----
                    p3 = cb.tile([128, 30, 30], BF16, tag="p3")
                    nc.vector.memset(p3[:, 0, :], 0.0)
                    nc.vector.memset(p3[:, 29, :], 0.0)
                    nc.vector.memset(p3[:, :, 0], 0.0)
                    nc.vector.memset(p3[:, :, 29], 0.0)
                    for c in range(4):
                        ics = [cw.tile([128, 14, 56], BF16, tag=f"ic3_{q}", name=f"ic3_{q}")
                               for q in range(5)]
                        for kk in range(9):
                            ky, kx = kk // 3, kk % 3
                            q, g = kk // 2, kk % 2
                            nc.sync.dma_start(
                                ics[q][64 * g:64 * g + 64, :, :],
                                p2[:, ky + 14 * c:ky + 14 * c + 14, kx:kx + 56])
                        for n in range(7):
                            ps = cps.tile([128, 112], F32, tag="cps")
                            for q in range(5):
                                kq = 128 if q < 4 else 64
                                nc.tensor.matmul(ps[:], w3sb[:kq, q, :],
                                                 ics[q][:kq, 2 * n:2 * n + 2, :],
                                                 start=(q == 0), stop=(q == 4))
                            ev = cw.tile([128, 56], F32, tag="ev3")
                            nc.scalar.activation(ev[:], ps[:, 0::2], AF.Copy)
                            wm = cw.tile([128, 56], F32, tag="wm3")
                            nc.vector.tensor_tensor(wm[:], ev[:], ps[:, 1::2],
                                                    op=ALU.max)
                            hm = cw.tile([128, 28], F32, tag="hm3")
                            nc.vector.tensor_tensor(hm[:], wm[:, 0:28], wm[:, 28:56],
                                                    op=ALU.max)
                            nc.scalar.activation(p3[:, 1 + 7 * c + n, 1:29], hm[:],
                                                 AF.Relu, bias=bc3s[:])

                    # ---- conv4: [128,28,28] -> [256,14,14] ----
                    fm = [cb.tile([128, 14, 14], BF16, tag=f"feat_{m}_{i}", name=f"feat_{m}_{i}")
                          for m in range(2)]
                    feat[0][i], feat[1][i] = fm
                    for c in range(2):
                        ics = [cw.tile([128, 14, 28], BF16, tag=f"ic4_{kk}", name=f"ic4_{kk}")
                               for kk in range(9)]
                        for kk in range(9):
                            ky, kx = kk // 3, kk % 3
                            nc.sync.dma_start(
                                ics[kk][:],
                                p3[:, ky + 14 * c:ky + 14 * c + 14, kx:kx + 28])
                        for m in range(2):
                            ps = cps.tile([128, 392], F32, tag="cps")
                            for kk in range(9):
                                nc.tensor.matmul(ps[:], w4sb[:, kk, 128 * m:128 * m + 128],
                                                 ics[kk][:], start=(kk == 0),
                                                 stop=(kk == 8))
                            ev = cw.tile([128, 196], F32, tag="ev4")
                            nc.scalar.activation(ev[:], ps[:, 0::2], AF.Copy)
                            wm = cw.tile([128, 196], F32, tag="wm4")
                            nc.vector.tensor_tensor(wm[:], ev[:], ps[:, 1::2],
                                                    op=ALU.max)
                            wmv = wm[:].rearrange("p (h w) -> p h w", h=14)
                            hm = cw.tile([128, 7, 14], F32, tag="hm4")
                            nc.vector.tensor_tensor(hm[:], wmv[:, 0::2, :],
                                                    wmv[:, 1::2, :], op=ALU.max)
                            nc.scalar.activation(fm[m][:, 7 * c:7 * c + 7, :], hm[:],
                                                 AF.Relu, bias=bc4s[:, m:m + 1])

            # ============ FC (K-sharded) ============
            a2a_in = dpool.tile([R, BL, 6272], BF16)
            a2a_out = dpool.tile([R, BL, 6272], BF16)
            for m in range(2):
                for i in range(BL):
                    for s in range(4):
                        nc.sync.dma_start(a2a_in[4 * m + s, i],
                                          feat[m][i][32 * s:32 * s + 32, :, :])
            nc.gpsimd.collective_compute(
                "AllToAll", ALU.bypass, replica_groups=[list(range(R))],
                ins=[a2a_in.opt()], outs=[a2a_out.opt()])

            with (
                tc.tile_pool(name="fcp", bufs=4) as fcp,
                tc.tile_pool(name="fps", bufs=1, space="PSUM") as fps,
                tc.tile_pool(name="gps", bufs=4, space="PSUM") as gps,
            ):
                fcx = res.tile([128, 49, 16], BF16)
                for j in range(R):
                    for i in range(BL):
                        nc.sync.dma_start(
                            fcx[:, :, 2 * j + i],
                            a2a_out[j, i].rearrange("(q p) -> p q", p=128))
                psfc = fps.tile([16, 512], F32)
                for q in range(49):
                    wt = fcp.tile([128, 512], BF16, tag="fcwt")
                    nc.sync.dma_start(wt[:], fcw[q])
                    nc.tensor.matmul(psfc[:], fcx[:, q, :], wt[:],
                                     start=(q == 0), stop=(q == 48))
                fc_part = res.tile([16, 512], F32)
                nc.scalar.activation(fc_part[:], psfc[:], AF.Copy)

                ar_in = dpool.tile([16, 512], F32)
                ar_out = dpool.tile([16, 512], F32)
                nc.sync.dma_start(ar_in[:], fc_part[:])
                nc.gpsimd.collective_compute(
                    "AllReduce", ALU.add, replica_groups=[list(range(R))],
                    ins=[ar_in.opt()], outs=[ar_out.opt()])
                imgfT = res.tile([128, 4, 16], F32)
                for q in range(4):
                    nc.sync.dma_start(
                        imgfT[:, q, :],
                        ar_out[:, 128 * q:128 * q + 128].transpose([1, 0]))
                imgfR = res.tile([128, 4, 16], BF16)
                for q in range(4):
                    nc.scalar.activation(imgfR[:, q, :], imgfT[:, q, :], AF.Relu,
                                         bias=fcbs[:, q:q + 1])
                if DEBUG_OUTS:
                    nc.sync.dma_start(dbg_if[:], imgfT[:])
                # broadcast img features over T into xT k-tiles 0..3
                for q in range(4):
                    dst = xT[:, q, :].rearrange("p (t b) -> p t b", t=T)
                    nc.vector.tensor_copy(
                        dst, imgfR[:, q, :].unsqueeze(1).broadcast_to([128, T, 16]))

                # ---- gates_x = X @ W_ih (+ biases), [128, t, 4x16] ----
                for m in range(4):
                    psg = gps.tile([128, 384], F32, tag="gpsx")
                    for k in range(12):
                        nc.tensor.matmul(psg[:], wihsb[:, 4 * k + m, :], xT[:, k, :],
                                         start=(k == 0), stop=(k == 11))
                    dst = gx[:, :, 16 * m:16 * m + 16]
                    nc.scalar.activation(
                        dst, psg[:].rearrange("p (t b) -> p t b", t=T),
                        AF.Identity, bias=bsums[:, m:m + 1])

            # ============ LSTM ============
            with (
                tc.tile_pool(name="lw", bufs=3) as lw,
                tc.tile_pool(name="lps", bufs=2, space="PSUM") as lps,
            ):
                c_t = lw.tile([128, 16], F32, tag="c")
                nc.vector.memset(c_t[:], 0.0)
                hT = lw.tile([128, 8, 16], BF16, tag="hT")
                nc.vector.memset(hT[:], 0.0)
                for t in range(T):
                    psg = lps.tile([128, 64], F32, tag="lpsg")
                    for m in range(4):
                        for k in range(8):
                            nc.tensor.matmul(psg[:, 16 * m:16 * m + 16],
                                             whhsb[:, 4 * k + m, :], hT[:, k, :],
                                             start=(k == 0), stop=(k == 7))
                    gsum = lw.tile([128, 64], F32, tag="gsum")
                    nc.vector.tensor_tensor(gsum[:], psg[:], gx[:, t, :], op=ALU.add)
                    acts = lw.tile([128, 64], F32, tag="acts")
                    nc.scalar.activation(acts[:, 0:48], gsum[:, 0:48], AF.Sigmoid)
                    nc.scalar.activation(acts[:, 48:64], gsum[:, 48:64], AF.Tanh)
                    t1 = lw.tile([128, 16], F32, tag="t1")
                    nc.vector.tensor_tensor(t1[:], acts[:, 0:16], acts[:, 48:64],
                                            op=ALU.mult)
                    t2 = lw.tile([128, 16], F32, tag="t2")
                    nc.vector.tensor_tensor(t2[:], acts[:, 16:32], c_t[:],
                                            op=ALU.mult)
                    c_t = lw.tile([128, 16], F32, tag="c")
                    nc.vector.tensor_tensor(c_t[:], t1[:], t2[:], op=ALU.add)
                    tcc = lw.tile([128, 16], F32, tag="tcc")
                    nc.scalar.activation(tcc[:], c_t[:], AF.Tanh)
                    h_sl = lw.tile([128, 16], BF16, tag="h_sl")
                    nc.vector.tensor_tensor(h_sl[:], acts[:, 32:48], tcc[:],
                                            op=ALU.mult)
                    ag_in = agd.tile([128, 16], BF16, tag="ag_in")
                    ag_out = agd.tile([R, 128, 16], BF16, tag="ag_out")
                    nc.sync.dma_start(ag_in[:], h_sl[:])
                    nc.gpsimd.collective_compute(
                        "AllGather", ALU.bypass, replica_groups=[list(range(R))],
                        ins=[ag_in.opt()], outs=[ag_out.opt()])
                    hT = lw.tile([128, 8, 16], BF16, tag="hT")
                    nc.sync.dma_start(hT[:],
                                      ag_out[:].rearrange("r p n -> p r n"))
                    nc.vector.tensor_copy(lstm_buf[:, :, 16 * t:16 * t + 16], hT[:])
                if DEBUG_OUTS:
                    hdbg = lw.tile([128, 8, 16], F32, tag="hdbg")
                    nc.vector.tensor_copy(hdbg[:], hT[:])
                    nc.sync.dma_start(dbg_h[:], hdbg[:])

            # ============ vocab projection ============
            with (
                tc.tile_pool(name="vw", bufs=3) as vw,
                tc.tile_pool(name="vps", bufs=4, space="PSUM") as vps,
            ):
                for m in range(10):
                    psv = vps.tile([128, 384], F32, tag="vps")
                    for k in range(8):
                        nc.tensor.matmul(psv[:], outwsb[:, 10 * k + m, :],
                                         lstm_buf[:, k, :],
                                         start=(k == 0), stop=(k == 7))
                    vout = vw.tile([128, 384], F32, tag="vout")
                    nc.scalar.activation(vout[:], psv[:], AF.Identity,
                                         bias=outbs[:, m:m + 1])
                    nc.sync.dma_start(logits[m], vout[:])

            if DEBUG_OUTS:
                for i in range(BL):
                    fdbg = res.tile([128, 196], F32, tag=f"fdbg{i}")
                    nc.vector.tensor_copy(fdbg[:], feat[0][i][:].rearrange(
                        "p h w -> p (h w)"))
                    nc.sync.dma_start(dbg_feat[i], fdbg[:])

    nc.compile()
    return nc


# --------------------------------------------------------------------------
# host-side prep
# --------------------------------------------------------------------------
def host_prep(inputs):
    f32 = np.float32
    ims = np.asarray(inputs['images'], f32)
    caps = np.asarray(inputs['captions'])
    emos = np.asarray(inputs['emotions'])

    def fold(i):
        w = np.asarray(inputs[f'w{i}'], f32)
        b = np.asarray(inputs[f'b{i}'], f32)
        g = np.asarray(inputs[f'g{i}'], f32)
        be = np.asarray(inputs[f'be{i}'], f32)
        m = np.asarray(inputs[f'm{i}'], f32)
        v = np.asarray(inputs[f'v{i}'], f32)
        s = g / np.sqrt(v + 1e-5)
        return w * s[:, None, None, None], (b - m) * s + be

    wf1, bf1 = fold(1)
    wf2, bf2 = fold(2)
    wf3, bf3 = fold(3)
    wf4, bf4 = fold(4)

    # conv1 block-diagonal [108, 128]: rows s*27 + (ky*9+kx*3+c), cols 32s+o
    w1bd = np.zeros((108, 128), f32)
    wm1 = wf1.transpose(2, 3, 1, 0).reshape(27, 32)   # [ky,kx,c -> j], o
    for s in range(4):
        w1bd[27 * s:27 * s + 27, 32 * s:32 * s + 32] = wm1
    bc1 = np.tile(bf1, 4)[:, None]

    def wmat(wf, cin, cout):
        # rows j = (ky*3+kx)*cin + c
        return wf.transpose(2, 3, 1, 0).reshape(9 * cin, cout)

    w2k = wmat(wf2, 32, 64).reshape(3, 96, 64)
    w3k = wmat(wf3, 64, 128)
    w4k = wmat(wf4, 128, 256)
    bc4 = bf4.reshape(2, 128).T.copy()

    fcwT = np.asarray(inputs['fc_w'], f32).T          # [50176, 512]
    fcb = np.asarray(inputs['fc_b'], f32).reshape(4, 128).T.copy()

    emo_feat = np.asarray(inputs['emo_emb'], f32)[emos]          # [16, 512]
    embs = np.asarray(inputs['tok_emb'], f32)[caps]              # [16, T, 512]
    xh = np.concatenate([
        np.broadcast_to(emo_feat.T[:, None, :], (512, T, B)).reshape(512, T * B),
        embs.transpose(2, 1, 0).reshape(512, T * B),
    ], axis=0)                                                   # [1024, 384]

    W_ih = np.asarray(inputs['W_ih'], f32)
    W_hh = np.asarray(inputs['W_hh'], f32)
    bsum_full = (np.asarray(inputs['b_ih'], f32) +
                 np.asarray(inputs['b_hh'], f32))
    out_wT = np.asarray(inputs['out_w'], f32).T                  # [1024, 10000]
    out_b = np.asarray(inputs['out_b'], f32)

    pad = np.pad(ims, ((0, 0), (0, 0), (1, 1), (1, 1)))

    in_maps = []
    for r in range(R):
        cols = [GPERM[m] * 1024 + r * 128 for m in range(4)]
        wih_r = np.empty((48, 128, 128), f32)
        whh_r = np.empty((32, 128, 128), f32)
        for k in range(12):
            for m in range(4):
                wih_r[4 * k + m] = W_ih[128 * k:128 * k + 128,
                                        cols[m]:cols[m] + 128]
        for k in range(8):
            for m in range(4):
                whh_r[4 * k + m] = W_hh[128 * k:128 * k + 128,
                                        cols[m]:cols[m] + 128]
        bsum_r = np.stack([bsum_full[cols[m]:cols[m] + 128]
                           for m in range(4)], axis=1)

        vsl = out_wT[:, 1250 * r:1250 * r + 1250]
        vpad = np.zeros((1024, 1280), f32)
        vpad[:, :1250] = vsl
        outw_r = np.empty((80, 128, 128), f32)
        for k in range(8):
            for m in range(10):
                outw_r[10 * k + m] = vpad[128 * k:128 * k + 128,
                                          128 * m:128 * m + 128]
        outb_r = np.zeros((1280,), f32)
        outb_r[:1250] = out_b[1250 * r:1250 * r + 1250]

        in_maps.append({
            "imgs": pad[BL * r:BL * r + BL].astype(BF),
            "w1bd": w1bd.astype(BF), "bc1": bc1.astype(f32),
            "w2k": w2k.astype(BF), "bc2": bf2[:, None].astype(f32),
            "w3k": w3k.astype(BF), "bc3": bf3[:, None].astype(f32),
            "w4k": w4k.astype(BF), "bc4": bc4.astype(f32),
            "fcw": fcwT[6272 * r:6272 * r + 6272].reshape(49, 128, 512).astype(BF),
            "fcb": fcb.astype(f32),
            "xh": xh.astype(BF),
            "wih": wih_r.astype(BF), "bsum": bsum_r.astype(f32),
            "whh": whh_r.astype(BF),
            "outw": outw_r.astype(BF),
            "outb": outb_r.reshape(10, 128).T.astype(f32).copy(),
        })
    return in_maps


def assemble(results):
    full = np.empty((B, T, 10000), np.float32)
    for r in range(R):
        lr = results[r]["logits"].reshape(1280, T, B)
        full[:, :, 1250 * r:1250 * r + 1250] = lr[:1250].transpose(2, 1, 0)
    return full


def get_runner():
    """Build (once) and return a callable: in_maps -> per-core results."""
    if 'runner' in _cache:
        return _cache['runner']
    nc = build_program()

    import jax
    from jax.sharding import Mesh, PartitionSpec
    from jax.experimental.shard_map import shard_map
    from concourse import bass2jax, mybir as _mb
    bass2jax.install_neuronx_cc_hook()

    partition_name = (nc.partition_id_tensor.name
                      if nc.partition_id_tensor else None)
    in_names, out_names, out_avals, zero_outs = [], [], [], []
    for alloc in nc.m.functions[0].allocations:
        if not isinstance(alloc, _mb.MemoryLocationSet):
            continue
        name = alloc.memorylocations[0].name
        if alloc.kind == "ExternalInput":
            if name != partition_name:
                in_names.append(name)
        elif alloc.kind == "ExternalOutput":
            out_names.append(name)
            np_dt = _mb.dt.np(alloc.dtype)
            out_avals.append(jax.core.ShapedArray(tuple(alloc.tensor_shape), np_dt))
            zero_outs.append(np.zeros(tuple(alloc.tensor_shape), np_dt))

    n_params = len(in_names)
    n_outs = len(out_names)
    all_in_names = list(in_names) + list(out_names)
    if partition_name is not None:
        all_in_names.append(partition_name)
    donate = tuple(range(n_params, n_params + n_outs))

    def _body(*args):
        operands = list(args)
        if partition_name is not None:
            operands.append(bass2jax.partition_id_tensor())
        outs = bass2jax._bass_exec_p.bind(
            *operands,
            out_avals=tuple(out_avals),
            in_names=tuple(all_in_names),
            out_names=tuple(out_names),
            lowering_input_output_aliases=(),
            sim_require_finite=True,
            sim_require_nnan=True,
            nc=nc,
        )
        return tuple(outs)

    devices = jax.devices()[:R]
    mesh = Mesh(np.asarray(devices), ("core",))
    in_specs = (PartitionSpec("core"),) * (n_params + n_outs)
    out_specs = (PartitionSpec("core"),) * n_outs
    sharded = jax.jit(
        shard_map(_body, mesh=mesh, in_specs=in_specs, out_specs=out_specs,
                  check_rep=False),
        donate_argnums=donate, keep_unused=True)

    def run(in_maps, device_inputs=None):
        if device_inputs is None:
            device_inputs = put_inputs(in_maps)
        concat_zeros = [np.zeros((R * z.shape[0], *z.shape[1:]), z.dtype)
                        for z in zero_outs]
        out_arrs = sharded(*device_inputs, *concat_zeros)
        return [
            {name: np.asarray(out_arrs[i]).reshape(R, *out_avals[i].shape)[c]
             for i, name in enumerate(out_names)}
            for c in range(R)
        ]

    def put_inputs(in_maps):
        return [np.concatenate([np.asarray(in_maps[c][n]) for c in range(R)],
                               axis=0) for n in in_names]

    run.put_inputs = put_inputs
    _cache['runner'] = run
    return run


def kernel(**inputs):
    run = get_runner()
    in_maps = host_prep(inputs)
    return assemble(run(in_maps))


# revision 23
# speedup vs baseline: 4153.7577x; 4153.7577x over previous
"""Trainium2 Bass kernel for CustomImageCaptionModel.

CNN encoder (4x conv3x3+BN+ReLU+pool) -> FC -> [ctx|emb] -> LSTM(T=24) ->
vocab projection.  8-core SPMD:
  - convs data-parallel over batch (2 images/core)
  - FC K-sharded (AllToAll features, AllReduce partials)
  - LSTM gate-sharded (512 gate-dims/core, per-step AllGather of h)
  - vocab projection sharded over vocab (1250 rows/core, batched over tokens)
"""
import sys
sys.path.insert(0, '/opt/trn_rl_repo')

import numpy as np
import ml_dtypes

import concourse.bass as bass
import concourse.bacc as bacc
import concourse.tile as tile
import concourse.mybir as mybir
from concourse import bass_utils

BF = ml_dtypes.bfloat16
F32 = mybir.dt.float32
BF16 = mybir.dt.bfloat16
AF = mybir.ActivationFunctionType
ALU = mybir.AluOpType

R = 8            # cores
BL = 2           # images per core
T = 24
B = 16
GPERM = [0, 1, 3, 2]   # gate order [i, f, o, g] (reference order i,f,g,o)

DEBUG_OUTS = False

_cache = {}


class _StageDone(Exception):
    def __init__(self, nc):
        self.nc = nc


# --------------------------------------------------------------------------
# device program
# --------------------------------------------------------------------------
def build_program(stage=4, local_cc=False):
    # stage: 1=convs+A2A, 2=+FC+gates_x, 3=+LSTM, 4=+vocab (full)
    # local_cc: replace collectives with local DMA stand-ins (timing sims)
    if stage == 5:
        stage, local_cc = 4, True
    nc = bacc.Bacc("TRN2", target_bir_lowering=False, debug=False,
                   enable_asserts=True, num_devices=R)

    d = {}
    def din(name, shape, dt):
        d[name] = nc.dram_tensor(name, list(shape), dt, kind="ExternalInput")
        return d[name]

    ic1h = din("ic1h", [BL, 108, 56, 224], BF16)
    w1bd = din("w1bd", [108, 128], BF16)
    bc1 = din("bc1", [128, 1], F32)
    w2k = din("w2k", [3, 96, 64], BF16)       # [ky][kx*32+c, o]
    bc2 = din("bc2", [64, 1], F32)
    w3a = din("w3a", [3, 128, 128], BF16)     # [ky][kx(0,1)*64+c, o]
    w3b = din("w3b", [3, 64, 128], BF16)      # [ky][c, o] for kx=2
    bc3 = din("bc3", [128, 1], F32)
    w4k = din("w4k", [9, 128, 256], BF16)     # [ky*3+kx][c, o]
    bc4 = din("bc4", [128, 2], F32)
    fcw = din("fcw", [49, 128, 512], BF16)
    fcb = din("fcb", [128, 4], F32)
    xh = din("xh", [1024, 384], BF16)
    wih = din("wih", [48, 128, 128], BF16)
    bsum = din("bsum", [128, 4], F32)
    whh = din("whh", [32, 128, 128], BF16)
    outw = din("outw", [80, 128, 128], BF16)
    outb = din("outb", [128, 10], F32)

    logits = nc.dram_tensor("logits", [10, 128, 384], F32, kind="ExternalOutput")
    if DEBUG_OUTS:
        dbg_if = nc.dram_tensor("dbg_if", [128, 4, 16], F32, kind="ExternalOutput")
        dbg_h = nc.dram_tensor("dbg_h", [128, 8, 16], F32, kind="ExternalOutput")
        dbg_feat = nc.dram_tensor("dbg_feat", [2, 128, 196], F32,
                                  kind="ExternalOutput")

    with tile.TileContext(nc) as tc:
        with (
            tc.tile_pool(name="res", bufs=1) as res,
            tc.tile_pool(name="dram", bufs=1, space="DRAM") as dpool,
            tc.tile_pool(name="agd", bufs=2, space="DRAM") as agd,
        ):
            # ---- resident weights ----
            w1sb = res.tile([108, 128], BF16)
            nc.sync.dma_start(w1sb[:], w1bd[:])
            w2sb = res.tile([96, 3, 64], BF16)
            nc.sync.dma_start(w2sb[:], w2k[:].transpose([1, 0, 2]))
            w3asb = res.tile([128, 3, 128], BF16)
            nc.sync.dma_start(w3asb[:], w3a[:].transpose([1, 0, 2]))
            w3bsb = res.tile([64, 3, 128], BF16)
            nc.sync.dma_start(w3bsb[:], w3b[:].transpose([1, 0, 2]))
            w4sb = res.tile([128, 9, 256], BF16)
            nc.sync.dma_start(w4sb[:], w4k[:].transpose([1, 0, 2]))
            bc1s = res.tile([128, 1], F32); nc.sync.dma_start(bc1s[:], bc1[:])
            bc2s = res.tile([64, 1], F32);  nc.sync.dma_start(bc2s[:], bc2[:])
            bc3s = res.tile([128, 1], F32); nc.sync.dma_start(bc3s[:], bc3[:])
            bc4s = res.tile([128, 2], F32); nc.sync.dma_start(bc4s[:], bc4[:])
            fcbs = res.tile([128, 4], F32); nc.sync.dma_start(fcbs[:], fcb[:])
            bsums = res.tile([128, 4], F32); nc.sync.dma_start(bsums[:], bsum[:])
            outbs = res.tile([128, 10], F32); nc.sync.dma_start(outbs[:], outb[:])

            # ---- persistent activations ----
            xT = res.tile([128, 12, 384], BF16)       # X^T k-tiles
            nc.sync.dma_start(xT[:, 4:12, :], xh[:].rearrange("(k p) n -> p k n", p=128))
            gx = res.tile([128, 24, 64], F32)         # precomputed x-gates (+bias)
            lstm_buf = res.tile([128, 8, 384], BF16)  # h^T for vocab projection
            feat = [[None] * BL for _ in range(2)]    # conv4 pooled output tiles

            # ============ conv encoder (per local image) ============
            with (
                tc.tile_pool(name="cwork", bufs=2) as cw,
                tc.tile_pool(name="cbig", bufs=1) as cb,
                tc.tile_pool(name="cpsum", bufs=2, space="PSUM") as cps,
            ):
                for i in range(BL):
                    # ---- conv1: [3,224,224] -> [32,112,112] ----
                    ic1 = cb.tile([108, 56, 224], BF16, tag="ic1")
                    nc.sync.dma_start(ic1[:], ic1h[i])
                    pr1 = cb.tile([128, 28, 112], BF16, tag="pr1")
                    for c in range(7):            # 8 strip-rows per chunk
                        ps = cps.tile([128, 4, 512], F32, tag="cps")
                        for n in range(4):
                            nc.tensor.matmul(
                                ps[:, n, 0:448], w1sb[:],
                                ic1[:, 8 * c + 2 * n:8 * c + 2 * n + 2, :],
                                start=True, stop=True)
                        ev = cw.tile([128, 4, 224], F32, tag="ev1")
                        nc.scalar.activation(ev[:], ps[:, :, 0:448:2], AF.Copy)
                        wm = cw.tile([128, 4, 224], F32, tag="wm1")
                        nc.vector.tensor_tensor(wm[:], ev[:], ps[:, :, 1:448:2],
                                                op=ALU.max)
                        hm = cw.tile([128, 4, 112], F32, tag="hm1")
                        nc.vector.tensor_tensor(hm[:], wm[:, :, 0:112],
                                                wm[:, :, 112:224], op=ALU.max)
                        nc.scalar.activation(pr1[:, 4 * c:4 * c + 4, :], hm[:],
                                             AF.Relu, bias=bc1s[:])
                    # shuffle strips -> p1 [32 ch, 114, 114] padded
                    p1 = cb.tile([32, 114, 114], BF16, tag="p1")
                    nc.vector.memset(p1[:, 0, :], 0.0)
                    nc.vector.memset(p1[:, 113, :], 0.0)
                    nc.vector.memset(p1[:, :, 0], 0.0)
                    nc.vector.memset(p1[:, :, 113], 0.0)
                    for s in range(4):
                        nc.sync.dma_start(p1[:, 28 * s + 1:28 * s + 29, 1:113],
                                          pr1[32 * s:32 * s + 32, :, :])

                    # ---- conv2: [32,112,112] -> [64,56,56] ----
                    p2 = cb.tile([64, 58, 58], BF16, tag="p2")
                    nc.vector.memset(p2[:, 0, :], 0.0)
                    nc.vector.memset(p2[:, 57, :], 0.0)
                    nc.vector.memset(p2[:, :, 0], 0.0)
                    nc.vector.memset(p2[:, :, 57], 0.0)
                    for c in range(7):
                        ic2 = cw.tile([96, 18, 112], BF16, tag="ic2")
                        for kx in range(3):
                            nc.sync.dma_start(
                                ic2[32 * kx:32 * kx + 32, :, :],
                                p1[:, 16 * c:16 * c + 18, kx:kx + 112])
                        ps = cps.tile([64, 4, 512], F32, tag="cps")
                        for n in range(4):
                            for ky in range(3):
                                nc.tensor.matmul(
                                    ps[:, n, 0:448], w2sb[:, ky, :],
                                    ic2[:, ky + 4 * n:ky + 4 * n + 4, :],
                                    start=(ky == 0), stop=(ky == 2))
                        ev = cw.tile([64, 4, 224], F32, tag="ev2")
                        nc.scalar.activation(ev[:], ps[:, :, 0:448:2], AF.Copy)
                        wm = cw.tile([64, 4, 224], F32, tag="wm2")
                        nc.vector.tensor_tensor(wm[:], ev[:], ps[:, :, 1:448:2],
                                                op=ALU.max)
                        wmv = wm[:].rearrange("p n (r w) -> p n r w", r=4)
                        hm = cw.tile([64, 4, 2, 56], F32, tag="hm2")
                        nc.vector.tensor_tensor(hm[:], wmv[:, :, 0::2, :],
                                                wmv[:, :, 1::2, :], op=ALU.max)
                        nc.scalar.activation(
                            p2[:, 1 + 8 * c:1 + 8 * c + 8, 1:57],
                            hm[:].rearrange("p n r w -> p (n r) w"),
                            AF.Relu, bias=bc2s[:])

                    # ---- conv3: [64,56,56] -> [128,28,28] ----
                    p3 = cb.tile([128, 30, 30], BF16, tag="p3")
                    nc.vector.memset(p3[:, 0, :], 0.0)
                    nc.vector.memset(p3[:, 29, :], 0.0)
                    nc.vector.memset(p3[:, :, 0], 0.0)
                    nc.vector.memset(p3[:, :, 29], 0.0)
                    for c, (r0, nr) in enumerate([(0, 32), (32, 24)]):
                        nb = nr // 8
                        ica = cw.tile([128, 34, 56], BF16, tag="ic3a")
                        icb = cw.tile([64, 34, 56], BF16, tag="ic3b")
                        for kx in range(2):
                            nc.sync.dma_start(
                                ica[64 * kx:64 * kx + 64, :nr + 2, :],
                                p2[:, r0:r0 + nr + 2, kx:kx + 56])
                        nc.sync.dma_start(icb[:, :nr + 2, :],
                                          p2[:, r0:r0 + nr + 2, 2:58])
                        ps = cps.tile([128, 4, 512], F32, tag="cps")
                        for n in range(nb):
                            for ky in range(3):
                                nc.tensor.matmul(
                                    ps[:, n, 0:448], w3asb[:, ky, :],
                                    ica[:, ky + 8 * n:ky + 8 * n + 8, :],
                                    start=(ky == 0), stop=False)
                                nc.tensor.matmul(
                                    ps[:, n, 0:448], w3bsb[:, ky, :],
                                    icb[:, ky + 8 * n:ky + 8 * n + 8, :],
                                    start=False, stop=(ky == 2))
                        ev = cw.tile([128, 4, 224], F32, tag="ev3")
                        nc.scalar.activation(ev[:, :nb, :], ps[:, :nb, 0:448:2], AF.Copy)
                        wm = cw.tile([128, 4, 224], F32, tag="wm3")
                        nc.vector.tensor_tensor(wm[:, :nb, :], ev[:, :nb, :],
                                                ps[:, :nb, 1:448:2], op=ALU.max)
                        wmv = wm[:].rearrange("p n (r w) -> p n r w", r=8)
                        hm = cw.tile([128, 4, 4, 28], F32, tag="hm3")
                        nc.vector.tensor_tensor(hm[:, :nb], wmv[:, :nb, 0::2, :],
                                                wmv[:, :nb, 1::2, :], op=ALU.max)
                        nc.scalar.activation(
                            p3[:, 1 + r0 // 2:1 + r0 // 2 + nr // 2, 1:29],
                            hm[:, :nb].rearrange("p n r w -> p (n r) w"),
                            AF.Relu, bias=bc3s[:])

                    # ---- conv4: [128,28,28] -> [256,14,14] ----
                    fm = [cb.tile([128, 14, 14], BF16, tag=f"feat_{m}_{i}", name=f"feat_{m}_{i}")
                          for m in range(2)]
                    feat[0][i], feat[1][i] = fm
                    ics = [cw.tile([128, 30, 28], BF16, tag=f"ic4_{kx}", name=f"ic4_{kx}", bufs=1)
                           for kx in range(3)]
                    for kx in range(3):
                        nc.sync.dma_start(ics[kx][:],
                                          p3[:, 0:30, kx:kx + 28])
                    for m in range(2):
                        ps = cps.tile([128, 2, 512], F32, tag="cps")
                        for n, (nr0, nnr) in enumerate([(0, 16), (16, 12)]):
                            for kk in range(9):
                                ky, kx = kk // 3, kk % 3
                                nc.tensor.matmul(
                                    ps[:, n, 0:28 * nnr],
                                    w4sb[:, kk, 128 * m:128 * m + 128],
                                    ics[kx][:, ky + nr0:ky + nr0 + nnr, :],
                                    start=(kk == 0), stop=(kk == 8))
                        for n, (nr0, nnr) in enumerate([(0, 16), (16, 12)]):
                            W = 28 * nnr
                            ev = cw.tile([128, 224], F32, tag="ev4")
                            nc.scalar.activation(ev[:, :W // 2], ps[:, n, 0:W:2],
                                                 AF.Copy)
                            wm = cw.tile([128, 224], F32, tag="wm4")
                            nc.vector.tensor_tensor(wm[:, :W // 2], ev[:, :W // 2],
                                                    ps[:, n, 1:W:2], op=ALU.max)
                            wmv = wm[:, 0:W // 2].rearrange("p (r w) -> p r w", w=14)
                            hm = cw.tile([128, 8, 14], F32, tag="hm4")
                            nc.vector.tensor_tensor(hm[:, :nnr // 2, :],
                                                    wmv[:, 0::2, :], wmv[:, 1::2, :],
                                                    op=ALU.max)
                            nc.scalar.activation(
                                fm[m][:, nr0 // 2:nr0 // 2 + nnr // 2, :],
                                hm[:, :nnr // 2, :], AF.Relu, bias=bc4s[:, m:m + 1])

            # ============ FC (K-sharded) ============
            a2a_in = dpool.tile([R, BL, 6272], BF16)
            a2a_out = dpool.tile([R, BL, 6272], BF16)
            for m in range(2):
                for i in range(BL):
                    for s in range(4):
                        nc.sync.dma_start(a2a_in[4 * m + s, i],
                                          feat[m][i][32 * s:32 * s + 32, :, :])
            if local_cc:
                nc.sync.dma_start(a2a_out[:], a2a_in[:])
            else:
                nc.gpsimd.collective_compute(
                    "AllToAll", ALU.bypass, replica_groups=[list(range(R))],
                    ins=[a2a_in.opt()], outs=[a2a_out.opt()])

            with (
                tc.tile_pool(name="fcp", bufs=4) as fcp,
                tc.tile_pool(name="post", bufs=1) as post,
                tc.tile_pool(name="fps", bufs=1, space="PSUM") as fps,
                tc.tile_pool(name="gps", bufs=2, space="PSUM") as gps,
            ):
                wihsb = post.tile([128, 48, 128], BF16)
                nc.sync.dma_start(wihsb[:], wih[:].transpose([1, 0, 2]))
                whhsb = post.tile([128, 32, 128], BF16)
                nc.sync.dma_start(whhsb[:], whh[:].transpose([1, 0, 2]))
                outwsb = post.tile([128, 80, 128], BF16)
                nc.sync.dma_start(outwsb[:], outw[:].transpose([1, 0, 2]))
                fcx = res.tile([128, 49, 16], BF16)
                for j in range(R):
                    for i in range(BL):
                        nc.sync.dma_start(
                            fcx[:, :, 2 * j + i],
                            a2a_out[j, i].rearrange("(q p) -> p q", p=128))
                if stage == 1:
                    dum = fcp.tile([128, 16], F32, tag="dum")
                    nc.vector.tensor_copy(dum[:], fcx[:, 0, :])
                    nc.sync.dma_start(logits[0, :, 0:16], dum[:])
                psfc = fps.tile([16, 512], F32)
                for q in range(49 if stage >= 2 else 0):
                    wt = fcp.tile([128, 512], BF16, tag="fcwt")
                    nc.sync.dma_start(wt[:], fcw[q])
                    nc.tensor.matmul(psfc[:], fcx[:, q, :], wt[:],
                                     start=(q == 0), stop=(q == 48))
                if stage >= 2:
                    fc_part = res.tile([16, 512], F32)
                    nc.scalar.activation(fc_part[:], psfc[:], AF.Copy)

                    ar_in = dpool.tile([16, 512], F32)
                    ar_out = dpool.tile([16, 512], F32)
                    nc.sync.dma_start(ar_in[:], fc_part[:])
                    if local_cc:
                        nc.sync.dma_start(ar_out[:], ar_in[:])
                    else:
                        nc.gpsimd.collective_compute(
                            "AllReduce", ALU.add, replica_groups=[list(range(R))],
                            ins=[ar_in.opt()], outs=[ar_out.opt()])
                    imgfT = res.tile([128, 4, 16], F32)
                    for q in range(4):
                        nc.sync.dma_start(
                            imgfT[:, q, :],
                            ar_out[:, 128 * q:128 * q + 128].transpose([1, 0]))
                    imgfR = res.tile([128, 4, 16], BF16)
                    for q in range(4):
                        nc.scalar.activation(imgfR[:, q, :], imgfT[:, q, :], AF.Relu,
                                             bias=fcbs[:, q:q + 1])
                    if DEBUG_OUTS:
                        nc.sync.dma_start(dbg_if[:], imgfT[:])
                    # broadcast img features over T into xT k-tiles 0..3
                    for q in range(4):
                        dst = xT[:, q, :].rearrange("p (t b) -> p t b", t=T)
                        nc.vector.tensor_copy(
                            dst, imgfR[:, q, :].unsqueeze(1).broadcast_to([128, T, 16]))

                # ---- gates_x = X @ W_ih (+ biases), [128, t, 4x16] ----
                for m in range(4 if stage >= 2 else 0):
                    psg = gps.tile([128, 384], F32, tag="gpsx")
                    for k in range(12):
                        nc.tensor.matmul(psg[:], wihsb[:, 4 * k + m, :], xT[:, k, :],
                                         start=(k == 0), stop=(k == 11))
                    dst = gx[:, :, 16 * m:16 * m + 16]
                    nc.scalar.activation(
                        dst, psg[:].rearrange("p (t b) -> p t b", t=T),
                        AF.Identity, bias=bsums[:, m:m + 1])

                if stage == 2:
                    with tc.tile_pool(name="dmp", bufs=1) as dmp:
                        dum = dmp.tile([128, 64], F32, tag="dum2")
                        nc.vector.tensor_copy(dum[:], gx[:, 0, :])
                        nc.sync.dma_start(logits[0, :, 0:64], dum[:])
                # ==== LSTM (+ interleaved vocab projection) ====
                with (
                    tc.tile_pool(name="lw", bufs=3) as lw,
                    tc.tile_pool(name="vw", bufs=2) as vw,
                    tc.tile_pool(name="lps", bufs=2, space="PSUM") as lps,
                    tc.tile_pool(name="vps", bufs=2, space="PSUM") as vps,
                ):
                    VB = 6
                    do_vocab = stage >= 4
                    c_t = lw.tile([128, 16], F32, tag="c")
                    nc.vector.memset(c_t[:], 0.0)
                    hT = lw.tile([128, 8, 16], BF16, tag="hT")
                    nc.vector.memset(hT[:], 0.0)
                    for t in range(T if stage >= 3 else 0):
                        psg = lps.tile([128, 64], F32, tag="lpsg")
                        for m in range(4):
                            for k in range(8):
                                nc.tensor.matmul(psg[:, 16 * m:16 * m + 16],
                                                 whhsb[:, 4 * k + m, :],
                                                 hT[:, k, :],
                                                 start=(k == 0), stop=(k == 7))
                        gsum = lw.tile([128, 64], F32, tag="gsum")
                        nc.vector.tensor_tensor(gsum[:], psg[:], gx[:, t, :],
                                                op=ALU.add)
                        acts = lw.tile([128, 64], F32, tag="acts")
                        nc.scalar.activation(acts[:, 0:48], gsum[:, 0:48],
                                             AF.Sigmoid)
                        nc.scalar.activation(acts[:, 48:64], gsum[:, 48:64],
                                             AF.Tanh)
                        t1 = lw.tile([128, 16], F32, tag="t1")
                        nc.vector.tensor_tensor(t1[:], acts[:, 0:16],
                                                acts[:, 48:64], op=ALU.mult)
                        t2 = lw.tile([128, 16], F32, tag="t2")
                        nc.vector.tensor_tensor(t2[:], acts[:, 16:32], c_t[:],
                                                op=ALU.mult)
                        c_t = lw.tile([128, 16], F32, tag="c")
                        nc.vector.tensor_tensor(c_t[:], t1[:], t2[:], op=ALU.add)
                        tcc = lw.tile([128, 16], F32, tag="tcc")
                        nc.scalar.activation(tcc[:], c_t[:], AF.Tanh)
                        h_sl = lw.tile([128, 16], BF16, tag="h_sl")
                        nc.vector.tensor_tensor(h_sl[:], acts[:, 32:48], tcc[:],
                                                op=ALU.mult)
                        ag_in = agd.tile([128, 16], BF16, tag="ag_in")
                        ag_out = agd.tile([R, 128, 16], BF16, tag="ag_out")
                        nc.sync.dma_start(ag_in[:], h_sl[:])
                        if local_cc:
                            nc.sync.dma_start(ag_out[0], ag_in[:])
                        else:
                            nc.gpsimd.collective_compute(
                                "AllGather", ALU.bypass,
                                replica_groups=[list(range(R))],
                                ins=[ag_in.opt()], outs=[ag_out.opt()])
                        hT = lw.tile([128, 8, 16], BF16, tag="hT")
                        nc.sync.dma_start(hT[:],
                                          ag_out[:].rearrange("r p n -> p r n"))
                        nc.vector.tensor_copy(
                            lstm_buf[:, :, 16 * t:16 * t + 16], hT[:])
                        # interleaved vocab projection for finished steps
                        if do_vocab and (t + 1) % VB == 0:
                            c0 = 16 * (t + 1 - VB)
                            c1 = 16 * (t + 1)
                            for m in range(10):
                                psv = vps.tile([128, 96], F32, tag="vps")
                                for k in range(8):
                                    nc.tensor.matmul(
                                        psv[:], outwsb[:, 10 * k + m, :],
                                        lstm_buf[:, k, c0:c1],
                                        start=(k == 0), stop=(k == 7))
                                vout = vw.tile([128, 96], F32, tag="vout")
                                nc.scalar.activation(vout[:], psv[:], AF.Identity,
                                                     bias=outbs[:, m:m + 1])
                                nc.sync.dma_start(logits[m, :, c0:c1], vout[:])
                    if DEBUG_OUTS and stage >= 3:
                        hdbg = lw.tile([128, 8, 16], F32, tag="hdbg")
                        nc.vector.tensor_copy(hdbg[:], hT[:])
                        nc.sync.dma_start(dbg_h[:], hdbg[:])
                if stage == 3:
                    with tc.tile_pool(name="dmp2", bufs=1) as dmp2:
                        dum3 = dmp2.tile([128, 384], F32, tag="dum3")
                        nc.vector.tensor_copy(dum3[:], lstm_buf[:, 0, :])
                        nc.sync.dma_start(logits[0], dum3[:])

            if DEBUG_OUTS:
                for i in range(BL):
                    fdbg = res.tile([128, 196], F32, tag=f"fdbg{i}")
                    nc.vector.tensor_copy(fdbg[:], feat[0][i][:].rearrange(
                        "p h w -> p (h w)"))
                    nc.sync.dma_start(dbg_feat[i], fdbg[:])

    nc.compile()
    return nc


# --------------------------------------------------------------------------
# host-side prep
# --------------------------------------------------------------------------
def host_prep(inputs):
    f32 = np.float32
    ims = np.asarray(inputs['images'], f32)
    caps = np.asarray(inputs['captions'])
    emos = np.asarray(inputs['emotions'])

    def fold(i):
        w = np.asarray(inputs[f'w{i}'], f32)
        b = np.asarray(inputs[f'b{i}'], f32)
        g = np.asarray(inputs[f'g{i}'], f32)
        be = np.asarray(inputs[f'be{i}'], f32)
        m = np.asarray(inputs[f'm{i}'], f32)
        v = np.asarray(inputs[f'v{i}'], f32)
        s = g / np.sqrt(v + 1e-5)
        return w * s[:, None, None, None], (b - m) * s + be

    wf1, bf1 = fold(1)
    wf2, bf2 = fold(2)
    wf3, bf3 = fold(3)
    wf4, bf4 = fold(4)

    # conv1 block-diagonal [108, 128]: rows s*27 + (ky*9+kx*3+c), cols 32s+o
    w1bd = np.zeros((108, 128), f32)
    wm1 = wf1.transpose(2, 3, 1, 0).reshape(27, 32)   # [ky,kx,c -> j], o
    for s in range(4):
        w1bd[27 * s:27 * s + 27, 32 * s:32 * s + 32] = wm1
    bc1 = np.tile(bf1, 4)[:, None]

    t2 = wf2.transpose(2, 3, 1, 0)     # [ky, kx, c, o]
    w2k = t2.reshape(3, 96, 64)
    t3 = wf3.transpose(2, 3, 1, 0)
    w3a = t3[:, 0:2].reshape(3, 128, 128).copy()
    w3b = t3[:, 2].copy()
    w4k = wf4.transpose(2, 3, 1, 0).reshape(9, 128, 256)
    bc4 = bf4.reshape(2, 128).T.copy()

    fcwT = np.asarray(inputs['fc_w'], f32).T          # [50176, 512]
    fcb = np.asarray(inputs['fc_b'], f32).reshape(4, 128).T.copy()

    emo_feat = np.asarray(inputs['emo_emb'], f32)[emos]          # [16, 512]
    embs = np.asarray(inputs['tok_emb'], f32)[caps]              # [16, T, 512]
    xh = np.concatenate([
        np.broadcast_to(emo_feat.T[:, None, :], (512, T, B)).reshape(512, T * B),
        embs.transpose(2, 1, 0).reshape(512, T * B),
    ], axis=0)                                                   # [1024, 384]

    W_ih = np.asarray(inputs['W_ih'], f32)
    W_hh = np.asarray(inputs['W_hh'], f32)
    bsum_full = (np.asarray(inputs['b_ih'], f32) +
                 np.asarray(inputs['b_hh'], f32))
    out_wT = np.asarray(inputs['out_w'], f32).T                  # [1024, 10000]
    out_b = np.asarray(inputs['out_b'], f32)

    pad = np.pad(ims, ((0, 0), (0, 0), (1, 1), (1, 1)))
    ic1h = np.empty((B, 108, 56, 224), f32)
    for st in range(4):
        for ky in range(3):
            for kx in range(3):
                for c in range(3):
                    j = 27 * st + 9 * ky + 3 * kx + c
                    ic1h[:, j] = pad[:, c, 56 * st + ky:56 * st + ky + 56,
                                     kx:kx + 224]

    in_maps = []
    for r in range(R):
        cols = [GPERM[m] * 1024 + r * 128 for m in range(4)]
        wih_r = np.empty((48, 128, 128), f32)
        whh_r = np.empty((32, 128, 128), f32)
        for k in range(12):
            for m in range(4):
                wih_r[4 * k + m] = W_ih[128 * k:128 * k + 128,
                                        cols[m]:cols[m] + 128]
        for k in range(8):
            for m in range(4):
                whh_r[4 * k + m] = W_hh[128 * k:128 * k + 128,
                                        cols[m]:cols[m] + 128]
        bsum_r = np.stack([bsum_full[cols[m]:cols[m] + 128]
                           for m in range(4)], axis=1)

        vsl = out_wT[:, 1250 * r:1250 * r + 1250]
        vpad = np.zeros((1024, 1280), f32)
        vpad[:, :1250] = vsl
        outw_r = np.empty((80, 128, 128), f32)
        for k in range(8):
            for m in range(10):
                outw_r[10 * k + m] = vpad[128 * k:128 * k + 128,
                                          128 * m:128 * m + 128]
        outb_r = np.zeros((1280,), f32)
        outb_r[:1250] = out_b[1250 * r:1250 * r + 1250]

        in_maps.append({
            "ic1h": ic1h[BL * r:BL * r + BL].astype(BF),
            "w1bd": w1bd.astype(BF), "bc1": bc1.astype(f32),
            "w2k": w2k.astype(BF), "bc2": bf2[:, None].astype(f32),
            "w3a": w3a.astype(BF), "w3b": w3b.astype(BF),
            "bc3": bf3[:, None].astype(f32),
            "w4k": w4k.astype(BF), "bc4": bc4.astype(f32),
            "fcw": fcwT[6272 * r:6272 * r + 6272].reshape(49, 128, 512).astype(BF),
            "fcb": fcb.astype(f32),
            "xh": xh.astype(BF),
            "wih": wih_r.astype(BF), "bsum": bsum_r.astype(f32),
            "whh": whh_r.astype(BF),
            "outw": outw_r.astype(BF),
            "outb": outb_r.reshape(10, 128).T.astype(f32).copy(),
        })
    return in_maps


def assemble(results):
    full = np.empty((B, T, 10000), np.float32)
    for r in range(R):
        lr = results[r]["logits"].reshape(1280, T, B)
        full[:, :, 1250 * r:1250 * r + 1250] = lr[:1250].transpose(2, 1, 0)
    return full


def get_runner():
    """Build (once) and return a callable: in_maps -> per-core results."""
    if 'runner' in _cache:
        return _cache['runner']
    run = make_runner(build_program())
    _cache['runner'] = run
    return run


def make_runner(nc):
    import jax
    from jax.sharding import Mesh, PartitionSpec
    from jax.experimental.shard_map import shard_map
    from concourse import bass2jax, mybir as _mb
    bass2jax.install_neuronx_cc_hook()

    partition_name = (nc.partition_id_tensor.name
                      if nc.partition_id_tensor else None)
    in_names, out_names, out_avals, zero_outs = [], [], [], []
    for alloc in nc.m.functions[0].allocations:
        if not isinstance(alloc, _mb.MemoryLocationSet):
            continue
        name = alloc.memorylocations[0].name
        if alloc.kind == "ExternalInput":
            if name != partition_name:
                in_names.append(name)
        elif alloc.kind == "ExternalOutput":
            out_names.append(name)
            np_dt = _mb.dt.np(alloc.dtype)
            out_avals.append(jax.core.ShapedArray(tuple(alloc.tensor_shape), np_dt))
            zero_outs.append(np.zeros(tuple(alloc.tensor_shape), np_dt))

    n_params = len(in_names)
    n_outs = len(out_names)
    all_in_names = list(in_names) + list(out_names)
    if partition_name is not None:
        all_in_names.append(partition_name)
    donate = tuple(range(n_params, n_params + n_outs))

    def _body(*args):
        operands = list(args)
        if partition_name is not None:
            operands.append(bass2jax.partition_id_tensor())
        outs = bass2jax._bass_exec_p.bind(
            *operands,
            out_avals=tuple(out_avals),
            in_names=tuple(all_in_names),
            out_names=tuple(out_names),
            lowering_input_output_aliases=(),
            sim_require_finite=True,
            sim_require_nnan=True,
            nc=nc,
        )
        return tuple(outs)

    devices = jax.devices()[:R]
    mesh = Mesh(np.asarray(devices), ("core",))
    in_specs = (PartitionSpec("core"),) * (n_params + n_outs)
    out_specs = (PartitionSpec("core"),) * n_outs
    sharded = jax.jit(
        shard_map(_body, mesh=mesh, in_specs=in_specs, out_specs=out_specs,
                  check_rep=False),
        donate_argnums=donate, keep_unused=True)

    def run(in_maps, device_inputs=None):
        if device_inputs is None:
            device_inputs = put_inputs(in_maps)
        concat_zeros = [np.zeros((R * z.shape[0], *z.shape[1:]), z.dtype)
                        for z in zero_outs]
        out_arrs = sharded(*device_inputs, *concat_zeros)
        return [
            {name: np.asarray(out_arrs[i]).reshape(R, *out_avals[i].shape)[c]
             for i, name in enumerate(out_names)}
            for c in range(R)
        ]

    def put_inputs(in_maps):
        return [np.concatenate([np.asarray(in_maps[c][n]) for c in range(R)],
                               axis=0) for n in in_names]

    run.put_inputs = put_inputs
    run.sharded = sharded
    run.out_names = out_names
    run.out_avals = out_avals
    run.zero_outs = zero_outs
    return run


def kernel(**inputs):
    run = get_runner()
    in_maps = host_prep(inputs)
    return assemble(run(in_maps))
